# revision 8
# baseline (speedup 1.0000x reference)
"""Trainium2 Bass kernel for nn_DifferentiableFDN.

Math: the module is linear in x, so
    out[b,t] = sum_j w_j * y_j[b,t],   w = (H^T alpha + beta)/16,
    y_j = first-order IIR of x with decay a_j.

Blocked-scan scheme (chunk length L=128, NCH=375 chunks per batch row,
chunk-tiles of 125 chunks):
  - local part (per chunk, j-summed):
        zT[c,tp] = sum_t Xc[c,t] * MT[t,tp],  MT[t,tp] = h[tp-t], h[d] = sum_j w_j a_j^d
  - chunk-end states: e[j,c] = sum_t a_j^(L-1-t) Xc[c,t]  (matmul with P),
        S[c] = a_j^L S[c-1] + e[c]  (DVE tensor_tensor_scan over chunks,
        written directly into the shifted position)
  - rank-16 carry correction accumulated into the same PSUM:
        zT[c,tp] += sum_j S[j,c-1] * (w_j a_j^(tp+1))     (matmul with Wc)

Sharding: pure data-parallel, 4 batch rows per core x 8 cores.
"""
import numpy as np

B, T = 32, 48000
D = 16
NCORES = 8
BL = B // NCORES            # 4 batch rows per core
L = 128                     # chunk length
NCH = T // L                # 375 chunks per batch row
ROWS = BL * NCH             # 1500 dram rows per core
CT = 125                    # chunk-tile: 3 uniform tiles of 125 chunks
NG = NCH // CT              # 3

# float32r (single-pass PE fp32) is ~2-4x faster but TF32-like: the BIR
# verifier requires producers to round operands to fp32r precision, which
# would push output error from ~1.6e-7 to ~5e-4. Keep exact fp32.
USE_FP32R = False

_CACHE = {}


def _mirror_f32_params(log_kappa, alpha_raw, beta_raw, H):
    """Reference param math, f64 internally, rounded through f32 where the
    reference's f32 pipeline rounds."""
    sig = 1.0 / (1.0 + np.exp(-log_kappa.astype(np.float64)))
    sig32 = sig.astype(np.float32)
    kappa = (np.float32(1.0) + sig32 * np.float32(799.0)).astype(np.float32)
    inv = (np.float32(-1.0) / kappa).astype(np.float32)
    decays = np.exp(inv.astype(np.float64)).astype(np.float32)
    decays = np.clip(decays, 0.0, 0.9999).astype(np.float64)
    alpha = (1.0 / (1.0 + np.exp(-alpha_raw.astype(np.float64))))
    beta = (1.0 / (1.0 + np.exp(-beta_raw.astype(np.float64))))
    alpha = alpha.astype(np.float32).astype(np.float64)
    beta = beta.astype(np.float32).astype(np.float64)
    w = (H.astype(np.float64).T @ alpha + beta) / np.float64(D)
    return decays, w


def _tables(decays, w):
    delta = np.arange(L)
    pows = decays[None, :] ** delta[:, None]                   # [L, D] a_j^d
    h = pows @ w                                               # h[d]
    MT = np.zeros((L, L))
    for t in range(L):
        MT[t, t:] = h[: L - t]                                 # MT[t,tp]=h[tp-t]
    P = decays[None, :] ** (L - 1 - delta[:, None])            # [L, D]
    Wc = w[:, None] * decays[:, None] ** (delta[None, :] + 1)  # [D, L]
    ml = np.tile((decays ** L)[:, None], (1, NCH - 1))         # [D, NCH-1]
    f = np.float32
    ident = np.eye(L, dtype=f)
    # pack constants: c1 = [MT | P | ident] (128 x 272), c2 = [Wc | ml] (16 x 502)
    c1 = np.concatenate([MT.astype(f), P.astype(f), ident], axis=1)
    c2 = np.concatenate([Wc.astype(f), ml.astype(f)], axis=1)
    return np.ascontiguousarray(c1), np.ascontiguousarray(c2)


def _body(tc, o_ap, x_ap, c1_ap, c2_ap):
    from concourse import mybir
    from contextlib import ExitStack

    nc = tc.nc
    f32 = mybir.dt.float32
    rdt = mybir.dt.float32r if USE_FP32R else mybir.dt.float32

    def rd(ap):
        return ap.bitcast(rdt) if USE_FP32R else ap

    x_re = x_ap.rearrange("(bg p) t -> p bg t", p=CT)          # [125, 12, 128]
    o_re = o_ap.rearrange("(b g p) t -> p b g t", p=CT, g=NG)  # [125, 4, 3, 128]

    with ExitStack() as ctx:
        const = ctx.enter_context(tc.tile_pool(name="const", bufs=1))
        natp = ctx.enter_context(tc.tile_pool(name="nat", bufs=1))
        xtp = ctx.enter_context(tc.tile_pool(name="xt", bufs=1))
        sshp = ctx.enter_context(tc.tile_pool(name="sshp", bufs=1))
        stgp = ctx.enter_context(tc.tile_pool(name="stg", bufs=2))
        tpp = ctx.enter_context(tc.tile_pool(name="tp_ps", bufs=2, space="PSUM"))
        epp = ctx.enter_context(tc.tile_pool(name="e_ps", bufs=2, space="PSUM"))
        zpp = ctx.enter_context(tc.tile_pool(name="z_ps", bufs=3, space="PSUM"))

        c1 = const.tile([L, 272], f32, tag="c1")
        nc.sync.dma_start(c1[:, :], c1_ap[:, :])
        c2 = const.tile([D, 502], f32, tag="c2")
        nc.sync.dma_start(c2[:, :], c2_ap[:, :])
        mt_sb, p_sb, id_sb = c1[:, 0:128], c1[:, 128:144], c1[:, 144:272]
        wc_sb, ml_sb = c2[:, 0:128], c2[:, 128:502]

        nat = [natp.tile([CT, NG, L], f32, tag=f"nat{b}", name=f"nat{b}")
               for b in range(BL)]
        xt = [xtp.tile([L, NCH], f32, tag=f"xt{b}", name=f"xt{b}")
              for b in range(BL)]
        ssh = [sshp.tile([D, NCH], f32, tag=f"ssh{b}", name=f"ssh{b}")
               for b in range(BL)]

        for b in range(BL):
            nc.sync.dma_start(nat[b][:, :, :], x_re[:, b * NG:(b + 1) * NG, :])

        for b in range(BL):
            tp = tpp.tile([L, NCH], f32, tag="tp")
            for g in range(NG):
                nc.tensor.transpose(tp[:L, g * CT:(g + 1) * CT],
                                    nat[b][:, g, :], id_sb[:CT, :CT])
            nc.scalar.copy(xt[b][:, :], tp[:, :])
            e_ps = epp.tile([D, NCH], f32, tag="e")
            nc.tensor.matmul(e_ps[:, :], lhsT=rd(p_sb), rhs=rd(xt[b][:, :]),
                             start=True, stop=True)
            nc.vector.tensor_tensor_scan(
                ssh[b][:, 1:NCH], data0=ml_sb[:, :], data1=e_ps[:, 0:NCH - 1],
                initial=0.0, op0=mybir.AluOpType.mult, op1=mybir.AluOpType.add)
            nc.vector.memset(ssh[b][:, 0:1], 0.0)

        for b in range(BL):
            # one accumulation group per PSUM bank: start zeroes the whole 2KB
            # zero-region, so only the first matmul starts and the last stops
            z_ps = zpp.tile([CT, NG * L], f32, tag="z")
            for g in range(NG):
                nc.tensor.matmul(z_ps[:, g * L:(g + 1) * L],
                                 lhsT=rd(xt[b][:, g * CT:(g + 1) * CT]),
                                 rhs=rd(mt_sb), start=(g == 0), stop=False,
                                 skip_group_check=True)
            for g in range(NG):
                nc.tensor.matmul(z_ps[:, g * L:(g + 1) * L],
                                 lhsT=rd(ssh[b][:, g * CT:(g + 1) * CT]),
                                 rhs=rd(wc_sb), start=False, stop=(g == NG - 1),
                                 skip_group_check=True)
            stg = stgp.tile([CT, NG, L], f32, tag="stg")
            nc.scalar.copy(stg[:, :, :], z_ps[:, :].rearrange("p (g t) -> p g t", g=NG))
            nc.gpsimd.dma_start(o_re[:, b, :, :], stg[:, :, :])


def _build(num_devices=NCORES):
    import concourse.tile as tile
    from concourse import bacc, mybir

    f32 = mybir.dt.float32
    nc = bacc.Bacc("TRN2", target_bir_lowering=False, debug=False,
                   num_devices=num_devices)
    x_ap = nc.dram_tensor("x", [ROWS, L], f32, kind="ExternalInput").ap()
    c1_ap = nc.dram_tensor("c1", [L, 272], f32, kind="ExternalInput").ap()
    c2_ap = nc.dram_tensor("c2", [D, 502], f32, kind="ExternalInput").ap()
    o_ap = nc.dram_tensor("out", [ROWS, L], f32, kind="ExternalOutput").ap()

    with tile.TileContext(nc) as tc:
        _body(tc, o_ap, x_ap, c1_ap, c2_ap)
    nc.compile()
    return nc


def _in_maps(x, log_kappa, alpha_raw, beta_raw, H):
    decays, w = _mirror_f32_params(np.asarray(log_kappa), np.asarray(alpha_raw),
                                   np.asarray(beta_raw), np.asarray(H))
    c1, c2 = _tables(decays, w)
    x = np.ascontiguousarray(np.asarray(x), dtype=np.float32)
    maps = []
    for c in range(NCORES):
        xs = x[c * BL:(c + 1) * BL].reshape(ROWS, L)
        maps.append({"x": xs, "c1": c1, "c2": c2})
    return maps


def kernel(x, log_kappa, alpha_raw, beta_raw, H):
    from concourse import bass_utils

    if "nc" not in _CACHE:
        _CACHE["nc"] = _build()
    nc = _CACHE["nc"]
    maps = _in_maps(x, log_kappa, alpha_raw, beta_raw, H)
    res = bass_utils.run_bass_kernel_spmd(nc, maps, core_ids=list(range(NCORES)))
    out = np.concatenate([res.results[c]["out"].reshape(BL, T)
                          for c in range(NCORES)], axis=0)
    return out.astype(np.float32)


# revision 9
# speedup vs baseline: 1.1619x; 1.1619x over previous
"""Trainium2 Bass kernel for nn_DifferentiableFDN.

Math: the module is linear in x, so
    out[b,t] = sum_j w_j * y_j[b,t],   w = (H^T alpha + beta)/16,
    y_j = first-order IIR of x with decay a_j.

Blocked-scan scheme (chunk length L=128, NCH=375 chunks per batch row).
The host pre-transposes x into XT[b] = (t=128, c=375) and un-transposes the
output, so the device kernel is just 3 matmuls per batch row (stationary
weights, 375-wide moving operands) plus a 375-step DVE scan:
  - e  = P^T  @ XT   (16 x 375)   chunk-end state contributions
  - S  : tensor_tensor_scan over chunks, S[c] = a_j^L S[c-1] + e[c],
         written directly into the shifted position ssh[c] = S[c-1]
  - z  = MT^T @ XT   (128 x 375)  local Toeplitz part  (PSUM, start)
  - z += Wc^T @ ssh  (128 x 375)  rank-16 carry correction (PSUM, stop)
  out[b, c*128+tp] = z[tp, c]

Sharding: pure data-parallel, 4 batch rows per core x 8 cores.
"""
import numpy as np

B, T = 32, 48000
D = 16
NCORES = 8
BL = B // NCORES            # 4 batch rows per core
L = 128                     # chunk length
NCH = T // L                # 375 chunks per batch row

_CACHE = {}


def _mirror_f32_params(log_kappa, alpha_raw, beta_raw, H):
    """Reference param math, f64 internally, rounded through f32 where the
    reference's f32 pipeline rounds."""
    sig = 1.0 / (1.0 + np.exp(-log_kappa.astype(np.float64)))
    sig32 = sig.astype(np.float32)
    kappa = (np.float32(1.0) + sig32 * np.float32(799.0)).astype(np.float32)
    inv = (np.float32(-1.0) / kappa).astype(np.float32)
    decays = np.exp(inv.astype(np.float64)).astype(np.float32)
    decays = np.clip(decays, 0.0, 0.9999).astype(np.float64)
    alpha = (1.0 / (1.0 + np.exp(-alpha_raw.astype(np.float64))))
    beta = (1.0 / (1.0 + np.exp(-beta_raw.astype(np.float64))))
    alpha = alpha.astype(np.float32).astype(np.float64)
    beta = beta.astype(np.float32).astype(np.float64)
    w = (H.astype(np.float64).T @ alpha + beta) / np.float64(D)
    return decays, w


def _tables(decays, w):
    delta = np.arange(L)
    pows = decays[None, :] ** delta[:, None]                   # [L, D] a_j^d
    h = pows @ w                                               # h[d]
    MT = np.zeros((L, L))
    for t in range(L):
        MT[t, t:] = h[: L - t]                                 # MT[t,tp]=h[tp-t]
    P = decays[None, :] ** (L - 1 - delta[:, None])            # [L, D]
    Wc = w[:, None] * decays[:, None] ** (delta[None, :] + 1)  # [D, L]
    ml = np.tile((decays ** L)[:, None], (1, NCH - 1))         # [D, NCH-1]
    f = np.float32
    # pack constants: c1 = [MT | P] (128 x 144), c2 = [Wc | ml] (16 x 502)
    c1 = np.concatenate([MT.astype(f), P.astype(f)], axis=1)
    c2 = np.concatenate([Wc.astype(f), ml.astype(f)], axis=1)
    return np.ascontiguousarray(c1), np.ascontiguousarray(c2)


def _body(tc, o_ap, x_ap, c1_ap, c2_ap):
    from concourse import mybir
    from contextlib import ExitStack

    nc = tc.nc
    f32 = mybir.dt.float32

    with ExitStack() as ctx:
        const = ctx.enter_context(tc.tile_pool(name="const", bufs=1))
        xtp = ctx.enter_context(tc.tile_pool(name="xt", bufs=1))
        sshp = ctx.enter_context(tc.tile_pool(name="sshp", bufs=1))
        stgp = ctx.enter_context(tc.tile_pool(name="stg", bufs=2))
        epp = ctx.enter_context(tc.tile_pool(name="e_ps", bufs=2, space="PSUM"))
        zpp = ctx.enter_context(tc.tile_pool(name="z_ps", bufs=3, space="PSUM"))

        c1 = const.tile([L, 144], f32, tag="c1")
        nc.sync.dma_start(c1[:, :], c1_ap[:, :])
        c2 = const.tile([D, 502], f32, tag="c2")
        nc.sync.dma_start(c2[:, :], c2_ap[:, :])
        mt_sb, p_sb = c1[:, 0:128], c1[:, 128:144]
        wc_sb, ml_sb = c2[:, 0:128], c2[:, 128:502]

        xt = [xtp.tile([L, NCH], f32, tag=f"xt{b}", name=f"xt{b}")
              for b in range(BL)]
        ssh = [sshp.tile([D, NCH], f32, tag=f"ssh{b}", name=f"ssh{b}")
               for b in range(BL)]

        for b in range(BL):
            nc.sync.dma_start(xt[b][:, :], x_ap[b * L:(b + 1) * L, :])

        for b in range(BL):
            e_ps = epp.tile([D, NCH], f32, tag="e")
            nc.tensor.matmul(e_ps[:, :], lhsT=p_sb, rhs=xt[b][:, :],
                             start=True, stop=True)
            nc.vector.tensor_tensor_scan(
                ssh[b][:, 1:NCH], data0=ml_sb, data1=e_ps[:, 0:NCH - 1],
                initial=0.0, op0=mybir.AluOpType.mult, op1=mybir.AluOpType.add)
            nc.vector.memset(ssh[b][:, 0:1], 0.0)

            z_ps = zpp.tile([L, NCH], f32, tag="z")
            nc.tensor.matmul(z_ps[:, :], lhsT=mt_sb, rhs=xt[b][:, :],
                             start=True, stop=False, skip_group_check=True)
            nc.tensor.matmul(z_ps[:, :], lhsT=wc_sb, rhs=ssh[b][:, :],
                             start=False, stop=True, skip_group_check=True)

            stg = stgp.tile([L, NCH], f32, tag="stg")
            nc.scalar.copy(stg[:, :], z_ps[:, :])
            nc.sync.dma_start(o_ap[:, b * NCH:(b + 1) * NCH], stg[:, :])


def _build(num_devices=NCORES):
    import concourse.tile as tile
    from concourse import bacc, mybir

    f32 = mybir.dt.float32
    nc = bacc.Bacc("TRN2", target_bir_lowering=False, debug=False,
                   num_devices=num_devices)
    x_ap = nc.dram_tensor("x", [BL * L, NCH], f32, kind="ExternalInput").ap()
    c1_ap = nc.dram_tensor("c1", [L, 144], f32, kind="ExternalInput").ap()
    c2_ap = nc.dram_tensor("c2", [D, 502], f32, kind="ExternalInput").ap()
    o_ap = nc.dram_tensor("out", [L, BL * NCH], f32, kind="ExternalOutput").ap()

    with tile.TileContext(nc) as tc:
        _body(tc, o_ap, x_ap, c1_ap, c2_ap)
    nc.compile()
    return nc


def _in_maps(x, log_kappa, alpha_raw, beta_raw, H):
    decays, w = _mirror_f32_params(np.asarray(log_kappa), np.asarray(alpha_raw),
                                   np.asarray(beta_raw), np.asarray(H))
    c1, c2 = _tables(decays, w)
    x = np.ascontiguousarray(np.asarray(x), dtype=np.float32)
    # host pre-transpose: (B, T) -> per-core (BL*L, NCH) chunk-transposed
    xt_all = x.reshape(B, NCH, L).transpose(0, 2, 1)  # (B, L, NCH)
    maps = []
    for c in range(NCORES):
        xs = np.ascontiguousarray(xt_all[c * BL:(c + 1) * BL]).reshape(BL * L, NCH)
        maps.append({"x": xs, "c1": c1, "c2": c2})
    return maps


def _gather(results):
    # out dram per core: (L, BL*NCH) = [tp, (b, c)] -> (BL, T)
    outs = []
    for c in range(NCORES):
        arr = results[c]["out"].reshape(L, BL, NCH)
        outs.append(arr.transpose(1, 2, 0).reshape(BL, T))  # out[b, c*L+tp]
    return np.concatenate(outs, axis=0)


def kernel(x, log_kappa, alpha_raw, beta_raw, H):
    from concourse import bass_utils

    if "nc" not in _CACHE:
        _CACHE["nc"] = _build()
    nc = _CACHE["nc"]
    maps = _in_maps(x, log_kappa, alpha_raw, beta_raw, H)
    res = bass_utils.run_bass_kernel_spmd(nc, maps, core_ids=list(range(NCORES)))
    return _gather(res.results).astype(np.float32)


# revision 10
# speedup vs baseline: 1.4686x; 1.2640x over previous
"""Trainium2 Bass kernel for nn_DifferentiableFDN.

Math: the module is linear in x, so
    out[b,t] = sum_j w_j * y_j[b,t],   w = (H^T alpha + beta)/16,
    y_j = first-order IIR of x with decay a_j.

Blocked-scan scheme (chunk length L=128, NCH=375 chunks per batch row).
The host pre-transposes x into XT[b] = (t=128, c=375) and un-transposes the
output, so the device kernel is just 3 matmuls per batch row (stationary
weights, 375-wide moving operands) plus a 375-step DVE scan:
  - e  = P^T  @ XT   (16 x 375)   chunk-end state contributions
  - S  : tensor_tensor_scan over chunks, S[c] = a_j^L S[c-1] + e[c],
         written directly into the shifted position ssh[c] = S[c-1]
  - z  = MT^T @ XT   (128 x 375)  local Toeplitz part  (PSUM, start)
  - z += Wc^T @ ssh  (128 x 375)  rank-16 carry correction (PSUM, stop)
  out[b, c*128+tp] = z[tp, c]

Sharding: pure data-parallel, 4 batch rows per core x 8 cores.
"""
import numpy as np

B, T = 32, 48000
D = 16
NCORES = 8
BL = B // NCORES            # 4 batch rows per core
L = 128                     # chunk length
NCH = T // L                # 375 chunks per batch row

_CACHE = {}


def _mirror_f32_params(log_kappa, alpha_raw, beta_raw, H):
    """Reference param math, f64 internally, rounded through f32 where the
    reference's f32 pipeline rounds."""
    sig = 1.0 / (1.0 + np.exp(-log_kappa.astype(np.float64)))
    sig32 = sig.astype(np.float32)
    kappa = (np.float32(1.0) + sig32 * np.float32(799.0)).astype(np.float32)
    inv = (np.float32(-1.0) / kappa).astype(np.float32)
    decays = np.exp(inv.astype(np.float64)).astype(np.float32)
    decays = np.clip(decays, 0.0, 0.9999).astype(np.float64)
    alpha = (1.0 / (1.0 + np.exp(-alpha_raw.astype(np.float64))))
    beta = (1.0 / (1.0 + np.exp(-beta_raw.astype(np.float64))))
    alpha = alpha.astype(np.float32).astype(np.float64)
    beta = beta.astype(np.float32).astype(np.float64)
    w = (H.astype(np.float64).T @ alpha + beta) / np.float64(D)
    return decays, w


def _tables(decays, w):
    delta = np.arange(L)
    pows = decays[None, :] ** delta[:, None]                   # [L, D] a_j^d
    h = pows @ w                                               # h[d]
    MT = np.zeros((L, L))
    for t in range(L):
        MT[t, t:] = h[: L - t]                                 # MT[t,tp]=h[tp-t]
    P = decays[None, :] ** (L - 1 - delta[:, None])            # [L, D]
    Wc = w[:, None] * decays[:, None] ** (delta[None, :] + 1)  # [D, L]
    ml = np.tile((decays ** L)[:, None], (1, NCH - 1))         # [D, NCH-1]
    f = np.float32
    # pack constants: c1 = [MT | P] (128 x 144), c2 = [Wc | ml] (16 x 502)
    c1 = np.concatenate([MT.astype(f), P.astype(f)], axis=1)
    c2 = np.concatenate([Wc.astype(f), ml.astype(f)], axis=1)
    return np.ascontiguousarray(c1), np.ascontiguousarray(c2)


def _body(tc, o_ap, x_ap, c1_ap, c2_ap):
    from concourse import mybir
    from contextlib import ExitStack

    nc = tc.nc
    f32 = mybir.dt.float32

    bf16 = mybir.dt.bfloat16

    with ExitStack() as ctx:
        const = ctx.enter_context(tc.tile_pool(name="const", bufs=1))
        xtp = ctx.enter_context(tc.tile_pool(name="xt", bufs=1))
        sshp = ctx.enter_context(tc.tile_pool(name="sshp", bufs=1))
        stgp = ctx.enter_context(tc.tile_pool(name="stg", bufs=2))
        warmp = ctx.enter_context(tc.tile_pool(name="warm", bufs=1))
        epp = ctx.enter_context(tc.tile_pool(name="e_ps", bufs=2, space="PSUM"))
        zpp = ctx.enter_context(tc.tile_pool(name="z_ps", bufs=3, space="PSUM"))
        wpp = ctx.enter_context(tc.tile_pool(name="w_ps", bufs=1, space="PSUM"))

        # PE warm-up: HAM throttles the PE to K=4/8 until ~4us of sustained
        # activity; fill the DMA-wait window with dummy bf16 matmuls so the
        # real fp32 work runs at full clock.
        warm = warmp.tile([L, 512], bf16, tag="warm")
        nc.vector.memset(warm[:, :], 0.0)
        w_ps = wpp.tile([L, 512], f32, tag="wps")
        for _ in range(14):
            nc.tensor.matmul(w_ps[:, :], lhsT=warm[:, 0:128], rhs=warm[:, :],
                             start=True, stop=True)

        c1 = const.tile([L, 144], f32, tag="c1")
        nc.sync.dma_start(c1[:, :], c1_ap[:, :])
        c2 = const.tile([D, 502], f32, tag="c2")
        nc.scalar.dma_start(c2[:, :], c2_ap[:, :])
        mt_sb, p_sb = c1[:, 0:128], c1[:, 128:144]
        wc_sb, ml_sb = c2[:, 0:128], c2[:, 128:502]

        xt = [xtp.tile([L, NCH], f32, tag=f"xt{b}", name=f"xt{b}")
              for b in range(BL)]
        ssh = [sshp.tile([D, NCH], f32, tag=f"ssh{b}", name=f"ssh{b}")
               for b in range(BL)]

        for b in range(BL):
            eng = nc.sync if b % 2 == 0 else nc.scalar
            eng.dma_start(xt[b][:, :], x_ap[b * L:(b + 1) * L, :])

        for b in range(BL):
            e_ps = epp.tile([D, NCH], f32, tag="e")
            nc.tensor.matmul(e_ps[:, :], lhsT=p_sb, rhs=xt[b][:, :],
                             start=True, stop=True)
            nc.vector.tensor_tensor_scan(
                ssh[b][:, 1:NCH], data0=ml_sb, data1=e_ps[:, 0:NCH - 1],
                initial=0.0, op0=mybir.AluOpType.mult, op1=mybir.AluOpType.add)
            nc.vector.memset(ssh[b][:, 0:1], 0.0)

            z_ps = zpp.tile([L, NCH], f32, tag="z")
            nc.tensor.matmul(z_ps[:, :], lhsT=mt_sb, rhs=xt[b][:, :],
                             start=True, stop=False, skip_group_check=True)
            nc.tensor.matmul(z_ps[:, :], lhsT=wc_sb, rhs=ssh[b][:, :],
                             start=False, stop=True, skip_group_check=True)

            stg = stgp.tile([L, NCH], f32, tag="stg")
            nc.scalar.copy(stg[:, :], z_ps[:, :])
            nc.sync.dma_start(o_ap[:, b * NCH:(b + 1) * NCH], stg[:, :])


def _build(num_devices=NCORES):
    import concourse.tile as tile
    from concourse import bacc, mybir

    f32 = mybir.dt.float32
    nc = bacc.Bacc("TRN2", target_bir_lowering=False, debug=False,
                   num_devices=num_devices)
    x_ap = nc.dram_tensor("x", [BL * L, NCH], f32, kind="ExternalInput").ap()
    c1_ap = nc.dram_tensor("c1", [L, 144], f32, kind="ExternalInput").ap()
    c2_ap = nc.dram_tensor("c2", [D, 502], f32, kind="ExternalInput").ap()
    o_ap = nc.dram_tensor("out", [L, BL * NCH], f32, kind="ExternalOutput").ap()

    with tile.TileContext(nc) as tc:
        _body(tc, o_ap, x_ap, c1_ap, c2_ap)
    nc.compile()
    return nc


def _in_maps(x, log_kappa, alpha_raw, beta_raw, H):
    decays, w = _mirror_f32_params(np.asarray(log_kappa), np.asarray(alpha_raw),
                                   np.asarray(beta_raw), np.asarray(H))
    c1, c2 = _tables(decays, w)
    x = np.ascontiguousarray(np.asarray(x), dtype=np.float32)
    # host pre-transpose: (B, T) -> per-core (BL*L, NCH) chunk-transposed
    xt_all = x.reshape(B, NCH, L).transpose(0, 2, 1)  # (B, L, NCH)
    maps = []
    for c in range(NCORES):
        xs = np.ascontiguousarray(xt_all[c * BL:(c + 1) * BL]).reshape(BL * L, NCH)
        maps.append({"x": xs, "c1": c1, "c2": c2})
    return maps


def _gather(results):
    # out dram per core: (L, BL*NCH) = [tp, (b, c)] -> (BL, T)
    outs = []
    for c in range(NCORES):
        arr = results[c]["out"].reshape(L, BL, NCH)
        outs.append(arr.transpose(1, 2, 0).reshape(BL, T))  # out[b, c*L+tp]
    return np.concatenate(outs, axis=0)


def kernel(x, log_kappa, alpha_raw, beta_raw, H):
    from concourse import bass_utils

    if "nc" not in _CACHE:
        _CACHE["nc"] = _build()
    nc = _CACHE["nc"]
    maps = _in_maps(x, log_kappa, alpha_raw, beta_raw, H)
    res = bass_utils.run_bass_kernel_spmd(nc, maps, core_ids=list(range(NCORES)))
    return _gather(res.results).astype(np.float32)


# revision 13
# speedup vs baseline: 1.5540x; 1.0581x over previous
"""Trainium2 Bass kernel for nn_DifferentiableFDN.

Math: the module is linear in x, so
    out[b,t] = sum_j w_j * y_j[b,t],   w = (H^T alpha + beta)/16,
    y_j = first-order IIR of x with decay a_j.

Blocked-scan scheme (chunk length L=128, NCH=375 chunks per batch row).
The host pre-transposes x into XT[b] = (t=128, c=375) and un-transposes the
output, so the device kernel is just 3 matmuls per batch row (stationary
weights, 375-wide moving operands) plus a 375-step DVE scan:
  - e  = P^T  @ XT   (16 x 375)   chunk-end state contributions
  - S  : tensor_tensor_scan over chunks, S[c] = a_j^L S[c-1] + e[c],
         written directly into the shifted position ssh[c] = S[c-1]
  - z  = MT^T @ XT   (128 x 375)  local Toeplitz part  (PSUM, start)
  - z += Wc^T @ ssh  (128 x 375)  rank-16 carry correction (PSUM, stop)
  out[b, c*128+tp] = z[tp, c]

Sharding: pure data-parallel, 4 batch rows per core x 8 cores.
"""
import numpy as np

B, T = 32, 48000
D = 16
NCORES = 8
BL = B // NCORES            # 4 batch rows per core
L = 128                     # chunk length
NCH = T // L                # 375 chunks per batch row

_CACHE = {}


def _mirror_f32_params(log_kappa, alpha_raw, beta_raw, H):
    """Reference param math, f64 internally, rounded through f32 where the
    reference's f32 pipeline rounds."""
    sig = 1.0 / (1.0 + np.exp(-log_kappa.astype(np.float64)))
    sig32 = sig.astype(np.float32)
    kappa = (np.float32(1.0) + sig32 * np.float32(799.0)).astype(np.float32)
    inv = (np.float32(-1.0) / kappa).astype(np.float32)
    decays = np.exp(inv.astype(np.float64)).astype(np.float32)
    decays = np.clip(decays, 0.0, 0.9999).astype(np.float64)
    alpha = (1.0 / (1.0 + np.exp(-alpha_raw.astype(np.float64))))
    beta = (1.0 / (1.0 + np.exp(-beta_raw.astype(np.float64))))
    alpha = alpha.astype(np.float32).astype(np.float64)
    beta = beta.astype(np.float32).astype(np.float64)
    w = (H.astype(np.float64).T @ alpha + beta) / np.float64(D)
    return decays, w


def _tables(decays, w):
    delta = np.arange(L)
    pows = decays[None, :] ** delta[:, None]                   # [L, D] a_j^d
    h = pows @ w                                               # h[d]
    MT = np.zeros((L, L))
    for t in range(L):
        MT[t, t:] = h[: L - t]                                 # MT[t,tp]=h[tp-t]
    P = decays[None, :] ** (L - 1 - delta[:, None])            # [L, D]
    Wc = w[:, None] * decays[:, None] ** (delta[None, :] + 1)  # [D, L]
    ml = np.tile((decays ** L)[:, None], (1, NCH - 1))         # [D, NCH-1]
    f = np.float32
    # pack constants: c1 = [MT | P] (128 x 144), c2 = [Wc | ml] (16 x 502)
    c1 = np.concatenate([MT.astype(f), P.astype(f)], axis=1)
    c2 = np.concatenate([Wc.astype(f), ml.astype(f)], axis=1)
    return np.ascontiguousarray(c1), np.ascontiguousarray(c2)


def _body(tc, o_ap, x_ap, c1_ap, c2_ap):
    from concourse import mybir
    from contextlib import ExitStack

    nc = tc.nc
    f32 = mybir.dt.float32

    bf16 = mybir.dt.bfloat16

    with ExitStack() as ctx:
        const = ctx.enter_context(tc.tile_pool(name="const", bufs=1))
        xtp = ctx.enter_context(tc.tile_pool(name="xt", bufs=1))
        sshp = ctx.enter_context(tc.tile_pool(name="sshp", bufs=1))
        stgp = ctx.enter_context(tc.tile_pool(name="stg", bufs=2))
        warmp = ctx.enter_context(tc.tile_pool(name="warm", bufs=1))
        epp = ctx.enter_context(tc.tile_pool(name="e_ps", bufs=2, space="PSUM"))
        zpp = ctx.enter_context(tc.tile_pool(name="z_ps", bufs=3, space="PSUM"))
        wpp = ctx.enter_context(tc.tile_pool(name="w_ps", bufs=1, space="PSUM"))

        # PE warm-up: HAM throttles the PE to K=4/8 until ~4us of sustained
        # activity; fill the DMA-wait window with dummy bf16 matmuls so the
        # real fp32 work runs at full clock.
        warm = warmp.tile([L, 512], bf16, tag="warm")
        nc.vector.memset(warm[:, :], 0.0)
        w_ps = wpp.tile([L, 512], f32, tag="wps")
        for _ in range(6):
            nc.tensor.matmul(w_ps[:, :], lhsT=warm[:, 0:128], rhs=warm[:, :],
                             start=True, stop=True)

        c1 = const.tile([L, 144], f32, tag="c1")
        nc.sync.dma_start(c1[:, :], c1_ap[:, :])
        c2 = const.tile([D, 502], f32, tag="c2")
        nc.scalar.dma_start(c2[:, :], c2_ap[:, :])
        mt_sb, p_sb = c1[:, 0:128], c1[:, 128:144]
        wc_sb, ml_sb = c2[:, 0:128], c2[:, 128:502]

        xt = [xtp.tile([L, NCH], f32, tag=f"xt{b}", name=f"xt{b}")
              for b in range(BL)]
        ssh = [sshp.tile([D, NCH], f32, tag=f"ssh{b}", name=f"ssh{b}")
               for b in range(BL)]

        for b in range(BL):
            eng = nc.sync if b % 2 == 0 else nc.scalar
            eng.dma_start(xt[b][:, :], x_ap[b * L:(b + 1) * L, :])

        # chunk-end states first: E matmuls feed the DVE scans, which run
        # behind the remaining E's so the corr matmuls never stall on them
        for b in range(BL):
            e_ps = epp.tile([D, NCH], f32, tag="e")
            nc.tensor.matmul(e_ps[:, :], lhsT=p_sb, rhs=xt[b][:, :],
                             start=True, stop=True)
            nc.vector.tensor_tensor_scan(
                ssh[b][:, 1:NCH], data0=ml_sb, data1=e_ps[:, 0:NCH - 1],
                initial=0.0, op0=mybir.AluOpType.mult, op1=mybir.AluOpType.add)
            nc.vector.memset(ssh[b][:, 0:1], 0.0)

        for b in range(BL):
            z_ps = zpp.tile([L, NCH], f32, tag="z")
            nc.tensor.matmul(z_ps[:, :], lhsT=mt_sb, rhs=xt[b][:, :],
                             start=True, stop=False, skip_group_check=True)
            nc.tensor.matmul(z_ps[:, :], lhsT=wc_sb, rhs=ssh[b][:, :],
                             start=False, stop=True, skip_group_check=True)
            stg = stgp.tile([L, NCH], f32, tag="stg")
            nc.vector.tensor_copy(stg[:, :], z_ps[:, :])
            eng = nc.sync if b % 2 == 0 else nc.scalar
            eng.dma_start(o_ap[:, b * NCH:(b + 1) * NCH], stg[:, :])


def _build(num_devices=NCORES):
    import concourse.tile as tile
    from concourse import bacc, mybir

    f32 = mybir.dt.float32
    nc = bacc.Bacc("TRN2", target_bir_lowering=False, debug=False,
                   num_devices=num_devices)
    x_ap = nc.dram_tensor("x", [BL * L, NCH], f32, kind="ExternalInput").ap()
    c1_ap = nc.dram_tensor("c1", [L, 144], f32, kind="ExternalInput").ap()
    c2_ap = nc.dram_tensor("c2", [D, 502], f32, kind="ExternalInput").ap()
    o_ap = nc.dram_tensor("out", [L, BL * NCH], f32, kind="ExternalOutput").ap()

    with tile.TileContext(nc) as tc:
        _body(tc, o_ap, x_ap, c1_ap, c2_ap)
    nc.compile()
    return nc


def _in_maps(x, log_kappa, alpha_raw, beta_raw, H):
    decays, w = _mirror_f32_params(np.asarray(log_kappa), np.asarray(alpha_raw),
                                   np.asarray(beta_raw), np.asarray(H))
    c1, c2 = _tables(decays, w)
    x = np.ascontiguousarray(np.asarray(x), dtype=np.float32)
    # host pre-transpose: (B, T) -> per-core (BL*L, NCH) chunk-transposed
    xt_all = x.reshape(B, NCH, L).transpose(0, 2, 1)  # (B, L, NCH)
    maps = []
    for c in range(NCORES):
        xs = np.ascontiguousarray(xt_all[c * BL:(c + 1) * BL]).reshape(BL * L, NCH)
        maps.append({"x": xs, "c1": c1, "c2": c2})
    return maps


def _gather(results):
    # out dram per core: (L, BL*NCH) = [tp, (b, c)] -> (BL, T)
    outs = []
    for c in range(NCORES):
        arr = results[c]["out"].reshape(L, BL, NCH)
        outs.append(arr.transpose(1, 2, 0).reshape(BL, T))  # out[b, c*L+tp]
    return np.concatenate(outs, axis=0)


def kernel(x, log_kappa, alpha_raw, beta_raw, H):
    from concourse import bass_utils

    if "nc" not in _CACHE:
        _CACHE["nc"] = _build()
    nc = _CACHE["nc"]
    maps = _in_maps(x, log_kappa, alpha_raw, beta_raw, H)
    res = bass_utils.run_bass_kernel_spmd(nc, maps, core_ids=list(range(NCORES)))
    return _gather(res.results).astype(np.float32)
